# revision 13
# baseline (speedup 1.0000x reference)
"""ConvDeepSet Trainium2 kernel.

Computes, for each batch b:
    d2[n,m]   = (c[n] - t[m])^2                          (PE matmul, K small)
    w[n,m]    = exp(s * d2[n,m])                          (ACT exp, s = -0.5/scale^2)
    out1[c,m] = sum_n ctx[n,c] * w[n,m]                   (PE matmul, accumulate over n)
    density   = out1 row for the ones-channel
    conv_c    = out1 rows for feature channels
    res[m,o]  = W0[o]*density[m] + b[o]
                + (sum_c WT[c,o]*conv_c[m]) / (density[m] + 1e-8)
Sharded data-parallel over B across 8 NeuronCores (2 batches per core).
"""

import sys

if "/opt/trn_rl_repo" not in sys.path:
    sys.path.insert(0, "/opt/trn_rl_repo")

import numpy as np
import ml_dtypes

import concourse.bass as bass
import concourse.bacc as bacc
import concourse.tile as tile
import concourse.mybir as mybir
from concourse.bass_utils import run_bass_kernel_spmd

B, N, M, CIN, COUT = 16, 512, 1024, 7, 64
C = CIN + 1
N_CORES = 8
BPC = B // N_CORES  # batches per core
NT = N // 128       # n-tiles per batch
F32 = mybir.dt.float32
F32R = mybir.dt.float32r
F16 = mybir.dt.float16
BF16 = mybir.dt.bfloat16

# (lhsT-part, rhs-part) index pairs for the bf16 3-way-split cross terms of -2*c*t
_SPLIT_PAIRS = [(0, 0), (0, 1), (1, 0), (0, 2), (2, 0), (1, 1), (1, 2), (2, 1)]
SPLIT_K = 6 + len(_SPLIT_PAIRS)  # 3 (c^2 rows) + 3 (t^2 rows) + cross terms


def _build(svals, diff_mode, mm1_mode, epi_bcast, reps=1, diff_pack=True):
    """Build the SPMD Bass program. svals: tuple of per-group exp scales."""
    G = len(svals)
    KD = SPLIT_K if diff_mode == "split" else 3
    d_dt = BF16 if diff_mode == "split" else F32
    w_dt = {"f16": F16, "f32r": F32R, "f32": F32}[mm1_mode]

    nc = bacc.Bacc("TRN2", target_bir_lowering=False, debug=False,
                   num_devices=N_CORES)

    ctx_io_dt = F32 if mm1_mode == "f32r" else w_dt
    # packed diff layout: n-tile k sits at partition base 32*(k%2), pair k//2
    if diff_pack:
        L_d = nc.dram_tensor("L", [BPC, 32 + KD, NT // 2, 128], d_dt,
                             kind="ExternalInput")
        R_d = nc.dram_tensor("R", [BPC, 32 + KD, M], d_dt,
                             kind="ExternalInput")
    else:
        L_d = nc.dram_tensor("L", [BPC, KD, NT, 128], d_dt,
                             kind="ExternalInput")
        R_d = nc.dram_tensor("R", [BPC, KD, M], d_dt, kind="ExternalInput")
    ctx_d = nc.dram_tensor("ctx", [BPC, 128, G * NT * C], ctx_io_dt,
                           kind="ExternalInput")
    ones_d = nc.dram_tensor("ones", [1, M], F32, kind="ExternalInput")
    rb_d = nc.dram_tensor("rb", [C + 1, COUT], F32, kind="ExternalInput")
    ra_d = nc.dram_tensor("ra", [C + 1, COUT + 1], F32, kind="ExternalInput")
    out_d = nc.dram_tensor("out", [BPC, M, COUT], F32, kind="ExternalOutput")

    def mm_cast_mm1(ap):
        return ap.bitcast(F32R) if mm1_mode == "f32r" else ap

    with tile.TileContext(nc) as tc:
        with (
            tc.tile_pool(name="const", bufs=1) as constp,
            tc.tile_pool(name="inp", bufs=2) as inp,
            tc.tile_pool(name="wp", bufs=3) as wp,
            tc.tile_pool(name="o1p", bufs=2) as o1p,
            tc.tile_pool(name="resp", bufs=2) as resp,
            tc.tile_pool(name="rcp", bufs=2) as rcp,
            tc.tile_pool(name="dps", bufs=2, space=bass.MemorySpace.PSUM) as dps,
            tc.tile_pool(name="o1ps", bufs=1, space=bass.MemorySpace.PSUM) as o1ps,
            tc.tile_pool(name="aps", bufs=1, space=bass.MemorySpace.PSUM) as aps,
            tc.tile_pool(name="bps", bufs=1, space=bass.MemorySpace.PSUM) as bps,
        ):
            rb_t = constp.tile([C + 1, COUT], F32, tag="rb")
            nc.sync.dma_start(rb_t[:], rb_d.ap())
            ra_t = constp.tile([C + 1, COUT + 1], F32, tag="ra")
            nc.sync.dma_start(ra_t[:], ra_d.ap())

            def emit_phase1(j):
                lshape = ([32 + KD, NT // 2, 128] if diff_pack
                          else [KD, NT, 128])
                L_t = inp.tile(lshape, d_dt, tag="L")
                nc.sync.dma_start(L_t[:], L_d.ap()[j])
                R_t = inp.tile([32 + KD, M] if diff_pack else [KD, M],
                               d_dt, tag="R")
                nc.sync.dma_start(R_t[:], R_d.ap()[j])
                ctx_t = inp.tile([128, G, NT, C], ctx_io_dt, tag="ctx")
                nc.sync.dma_start(
                    ctx_t[:],
                    ctx_d.ap()[j].rearrange("p (g k c) -> p g k c",
                                            g=G, k=NT),
                )

                o1_t = o1ps.tile([C, M], F32, tag="o1")
                for k in range(NT):
                    base = 32 * (k % 2) if diff_pack else 0
                    d_t = dps.tile([128, M], F32, tag="d")
                    lhsT = (L_t[base:base + KD, k // 2, :] if diff_pack
                            else L_t[:, k, :])
                    for h in range(2):
                        nc.tensor.matmul(
                            d_t[:, h * 512:(h + 1) * 512],
                            lhsT,
                            R_t[base:base + KD, h * 512:(h + 1) * 512],
                            start=True, stop=True,
                            tile_position=(base, 0) if diff_pack else None,
                        )
                    for g in range(G):
                        w_t = wp.tile([128, M], w_dt, tag="w")
                        nc.scalar.activation(
                            w_t[:], d_t[:],
                            mybir.ActivationFunctionType.Exp,
                            scale=float(svals[g]),
                        )
                        first = (k == 0 and g == 0)
                        last = (k == NT - 1 and g == G - 1)
                        for h in range(2):
                            nc.tensor.matmul(
                                o1_t[:, h * 512:(h + 1) * 512],
                                mm_cast_mm1(ctx_t[:, g, k, :]),
                                mm_cast_mm1(w_t[:, h * 512:(h + 1) * 512]),
                                start=first, stop=last,
                            )
                return j, o1_t

            def emit_epilogue(j, o1_t):
                # division by density + final linear, m blocked as
                # m = 8*p + kk (partition p, group kk)
                o1_sb = o1p.tile([C + 1, M], F32, tag="o1sb")
                nc.vector.tensor_copy(o1_sb[0:C, :], o1_t[:])
                nc.sync.dma_start(o1_sb[C:C + 1, :], ones_d.ap())
                o1_g = o1_sb[:].rearrange("p (m q) -> p q m", q=8)

                res_t = resp.tile([128, 8 * COUT], F32, tag="res")
                for wave in range(2):
                    a_t = aps.tile([128, 4 * (COUT + 1)], F32, tag="a")
                    b_t = bps.tile([128, 4 * COUT], F32, tag="b")
                    for g4 in range(4):
                        kk = wave * 4 + g4
                        lhsT9 = o1_g[:, kk, :]
                        nc.tensor.matmul(
                            b_t[:, g4 * COUT:(g4 + 1) * COUT],
                            lhsT9, rb_t[:], start=True, stop=True,
                        )
                        nc.tensor.matmul(
                            a_t[:, g4 * (COUT + 1):(g4 + 1) * (COUT + 1)],
                            lhsT9, ra_t[:], start=True, stop=True,
                        )
                    a_g = a_t[:].rearrange("p (g x) -> p g x", x=COUT + 1)
                    recip_t = rcp.tile([128, 4], F32, tag="recip")
                    nc.vector.reciprocal(recip_t[:], a_g[:, :, COUT])
                    res_g = (res_t[:, wave * 4 * COUT:(wave + 1) * 4 * COUT]
                             .rearrange("p (g x) -> p g x", x=COUT))
                    if epi_bcast:
                        rb_ap = recip_t[:].unsqueeze(2).broadcast_to([128, 4, COUT])
                        nc.vector.tensor_tensor(
                            res_g, b_t[:].rearrange("p (g x) -> p g x", x=COUT),
                            rb_ap, mybir.AluOpType.mult,
                        )
                    else:
                        for g4 in range(4):
                            nc.vector.tensor_scalar_mul(
                                res_t[:, g4 * COUT:(g4 + 1) * COUT],
                                b_t[:, g4 * COUT:(g4 + 1) * COUT],
                                recip_t[:, g4:g4 + 1],
                            )
                    nc.vector.tensor_add(res_g, res_g, a_g[:, :, 0:COUT])
                nc.sync.dma_start(
                    out_d.ap()[j].rearrange("(p q) o -> p (q o)", q=8),
                    res_t[:],
                )

            pending = None
            for rep_j in range(reps * BPC):
                st = emit_phase1(rep_j % BPC)
                if pending is not None:
                    emit_epilogue(*pending)
                pending = st
            emit_epilogue(*pending)

    nc.compile()
    return nc


_CACHE = {}


def _get_program(svals, diff_mode, mm1_mode, epi_bcast, reps=1,
                 diff_pack=True):
    key = (tuple(np.float32(svals).tolist()), diff_mode, mm1_mode, epi_bcast,
           reps, diff_pack)
    if key not in _CACHE:
        _CACHE[key] = _build(svals, diff_mode, mm1_mode, epi_bcast, reps,
                             diff_pack)
    return _CACHE[key]


def _split3(x64):
    """Split float64 array into 3 bf16 arrays summing to ~fp32 accuracy."""
    parts = []
    r = x64.copy()
    for _ in range(3):
        p = r.astype(np.float32).astype(ml_dtypes.bfloat16)
        parts.append(p)
        r = r - p.astype(np.float64)
    return parts


def _host_prep(context_in, context_out, target_in, sigma, W, b,
               diff_mode, mm1_mode, diff_pack=True):
    ci = np.ascontiguousarray(np.asarray(context_in, np.float32)[:, :, 0])
    ti = np.ascontiguousarray(np.asarray(target_in, np.float32)[:, :, 0])
    co = np.asarray(context_out, np.float32)
    sig = np.asarray(sigma, np.float32)
    W = np.asarray(W, np.float32)
    bb = np.asarray(b, np.float32)

    scales = np.exp(sig.astype(np.float64))
    svals = (-0.5 / scales ** 2).astype(np.float32)
    uniq, inv = np.unique(svals, return_inverse=True)
    G = len(uniq)

    c64 = ci.astype(np.float64)
    t64 = ti.astype(np.float64)
    if diff_mode == "split":
        dt_np = ml_dtypes.bfloat16
        c_p = _split3(c64)
        t_p = _split3(t64)
        c2_p = _split3(c64 ** 2)
        t2_p = _split3(t64 ** 2)
        onesN = np.ones((B, N), dt_np)
        onesM = np.ones((B, M), dt_np)
        Lrows = c2_p + [onesN] * 3
        Rrows = [onesM] * 3 + t2_p
        for (i, jj) in _SPLIT_PAIRS:
            Lrows.append(c_p[i])
            Rrows.append((-2.0 * t_p[jj].astype(np.float32)).astype(dt_np))
        Lflat = np.stack(Lrows, axis=1)      # (B, SPLIT_K, N)
        Rflat = np.stack(Rrows, axis=1)      # (B, SPLIT_K, M)
        KD = SPLIT_K
    else:
        dt_np = np.float32
        Lflat = np.stack([c64 ** 2, -2.0 * c64, np.ones_like(c64)],
                         axis=1).astype(np.float32)
        Rflat = np.stack([np.ones_like(t64), t64, t64 ** 2],
                         axis=1).astype(np.float32)
        KD = 3
    # pack for row-group-concurrent diff matmuls: n-tile k at partition
    # base 32*(k%2), pair index k//2
    Lt = Lflat.reshape(B, KD, NT, 128)
    if diff_pack:
        L = np.zeros((B, 32 + KD, NT // 2, 128), dt_np)
        R = np.zeros((B, 32 + KD, M), dt_np)
        for k in range(NT):
            base = 32 * (k % 2)
            L[:, base:base + KD, k // 2, :] = Lt[:, :, k, :]
        R[:, 0:KD, :] = Rflat
        R[:, 32:32 + KD, :] = Rflat
    else:
        L = np.ascontiguousarray(Lt)
        R = np.ascontiguousarray(Rflat)

    w_np = np.float16 if mm1_mode == "f16" else np.float32
    ctx = np.zeros((B, G, N, C), w_np)
    for ch in range(C):
        g = int(inv[ch])
        if ch == 0:
            ctx[:, g, :, C - 1] = 1.0
        else:
            ctx[:, g, :, ch - 1] = co[:, :, ch - 1].astype(w_np)
    # device layout: partition p holds (g, k, c) contiguous
    ctx = np.ascontiguousarray(
        ctx.reshape(B, G, NT, 128, C).transpose(0, 3, 1, 2, 4)
        .reshape(B, 128, G * NT * C))

    # rb rows 0..6: W[:, 1:8].T ; rows 7,8: zero
    rb = np.zeros((C + 1, COUT), np.float32)
    rb[0:CIN, :] = W[:, 1:C].T
    # ra row 7: [W[:,0], 1]; row 8: [b, 1e-8]; rows 0..6: zero
    ra = np.zeros((C + 1, COUT + 1), np.float32)
    ra[C - 1, 0:COUT] = W[:, 0]
    ra[C - 1, COUT] = 1.0
    ra[C, 0:COUT] = bb
    ra[C, COUT] = 1e-8

    onesrow = np.ones((1, M), np.float32)

    in_maps = []
    for core in range(N_CORES):
        sl = slice(core * BPC, (core + 1) * BPC)
        in_maps.append({
            "L": np.ascontiguousarray(L[sl]),
            "R": np.ascontiguousarray(R[sl]),
            "ctx": np.ascontiguousarray(ctx[sl]),
            "ones": onesrow,
            "rb": rb,
            "ra": ra,
        })
    return uniq, in_maps


DIFF_MODE = "split"
MM1_MODE = "f16"
EPI_BCAST = True
DIFF_PACK = False


def kernel(context_in, context_out, target_in, sigma, W, b,
           diff_mode=None, mm1_mode=None, epi_bcast=None, trace=False,
           diff_pack=None):
    diff_mode = diff_mode or DIFF_MODE
    mm1_mode = mm1_mode or MM1_MODE
    epi_bcast = EPI_BCAST if epi_bcast is None else epi_bcast
    diff_pack = DIFF_PACK if diff_pack is None else diff_pack

    uniq_svals, in_maps = _host_prep(
        context_in, context_out, target_in, sigma, W, b, diff_mode, mm1_mode,
        diff_pack)
    nc = _get_program(tuple(uniq_svals.tolist()), diff_mode, mm1_mode,
                      epi_bcast, 1, diff_pack)
    res = run_bass_kernel_spmd(nc, in_maps, core_ids=list(range(N_CORES)),
                               trace=trace)
    out = np.concatenate([res.results[i]["out"] for i in range(N_CORES)],
                         axis=0)
    if trace:
        kernel.last_exec_time_ns = res.exec_time_ns
        kernel.last_results = res
    return out
